# revision 7
# baseline (speedup 1.0000x reference)
"""Trainium2 Bass kernel: DragonFly sparsity plugin (topk_masking).

Reference semantics (per batch sample, fp32):
  low  = x[:576].reshape(24, 24, 1024)   -> l2-normalize last dim
  high = x[576:].reshape(24, 96, 1024)   -> l2-normalize last dim
  q    = low_hat.mean(axis=1)            # [24, 1024]
  inner= einsum('pd,pgd->pg', q, high_hat)
  idx  = top_k(inner, 8)                 # [24, 8]
  out  = concat(low_hat.reshape(576, d), high_hat[p, idx].reshape(192, d))

Sharding: pure data parallel, 2 batch samples per core x 8 cores.

v2 layout: all big passes run on 128-token tiles. The per-token patch
query is broadcast on the PE via one-hot selector matmuls (emat), the
q.h dot is a single fused DVE tensor_tensor_reduce per tile, and the
per-token scores are regrouped to [24 patches, 96] through a tiny
PE-transpose + DRAM roundtrip before top-k.
"""

import numpy as np

import bass_rust
import concourse.bacc as bacc
import concourse.bass as bass
import concourse.tile as tile
from concourse import mybir
from concourse.bass import IndirectOffsetOnAxis
from concourse.bass_utils import run_bass_kernel_spmd


def _patch_tile_drain():
    """The walrus build in this image rejects instructions carrying >2 sync
    waits (CoreV3 setupSyncWait: "Too many sync wait commands"). Tile's
    end-of-kernel drain attaches one wait per live semaphore, so spread the
    waits over single-wait NOP carriers ahead of the drain instead."""
    if getattr(tile.TileContext, "_drain_patch_installed", False):
        return

    def patched(self, tick_clock, wait_clock):
        nc = self.nc
        probe = nc.sync.nop(nofuse=True)
        wait_clock.add_sem_waits(
            probe.ins, tile.ScopedClock({None: tick_clock.global_clock})
        )
        si = probe.ins.sync_info
        waits = list(si.on_wait) if si is not None else []
        if si is not None:
            si.on_wait = waits[:1]
        for i in range(1, len(waits)):
            n = nc.sync.nop(nofuse=True)
            n.ins.sync_info = bass_rust.SyncInfo(on_wait=[waits[i]], on_update=[])
        nc.sync.drain()
        nc.all_engine_barrier()
        popped = nc._tile_sem_poison_stack.pop()
        assert popped is self._sem_poison
        nc.clear_and_free_semaphores(list(self.sems.allocated().values()))
        nc.all_engine_barrier()

    tile.TileContext._drain_and_barrier = patched
    tile.TileContext._drain_patch_installed = True


_patch_tile_drain()

MAX_SYNC_WAITS = 2


def _split_excess_waits(nc, max_waits=MAX_SYNC_WAITS):
    """Walrus in this image caps sync waits per instruction; hoist excess
    waits onto single-wait NOPs queued just before the instruction on the
    same engine (identical blocking semantics)."""
    k = 0
    for f in nc.m.functions:
        for b in f.blocks:
            rewritten = []
            dirty = False
            for ins in b.instructions:
                si = ins.sync_info
                waits = list(si.on_wait) if si is not None else []
                n_upd = len(si.on_update) if si is not None else 0
                budget = max(max_waits - n_upd, 1 if waits else 0)
                if len(waits) > budget:
                    dirty = True
                    n_extra = len(waits) - budget
                    for j in range(n_extra):
                        n = mybir.InstNoOp(
                            name=f"I-wsplit-{k}", ins=[], outs=[], engine=ins.engine
                        )
                        k += 1
                        n.sync_info = bass_rust.SyncInfo(
                            on_wait=[waits[j]], on_update=[]
                        )
                        rewritten.append(n)
                    si.on_wait = waits[n_extra:]
                rewritten.append(ins)
            if dirty:
                b.instructions = rewritten


BSZ, SEQ, D = 16, 2880, 1024
N_LOW, N_HIGH = 576, 2304
P_PATCH = 24  # patches per sample
GL, GH = 24, 96  # low/high tokens per patch
TOP_K = 8
N_CORES = 8
SPC = BSZ // N_CORES  # samples per core
OUT_SEQ = N_LOW + P_PATCH * TOP_K  # 768
KT = N_HIGH // 128  # 18 high k-tiles per sample
HB = 3  # high k-tiles per DMA block
NHB = KT // HB  # 6 high DMA blocks per sample

F32 = mybir.dt.float32
U32 = mybir.dt.uint32
AF = mybir.ActivationFunctionType
OP = mybir.AluOpType


def host_constants():
    # gmat[i, t, p] = 1/24 if low token t*128+i belongs to patch p else 0
    g = np.zeros((128, 5, P_PATCH), np.float32)
    for t in range(5):
        for i in range(128):
            tok = t * 128 + i
            if tok < N_LOW:
                g[i, t, tok // GL, ] = 1.0 / GL
    # emat[p, k, i] = 1 if high token k*128+i belongs to patch p: one-hot
    # selector so E_k.T @ q lands q[patch(token)] on the token's partition
    e = np.zeros((P_PATCH, KT, 128), np.float32)
    for k in range(KT):
        for i in range(128):
            e[(k * 128 + i) // GH, k, i] = 1.0
    pbase = (N_LOW + GH * np.arange(P_PATCH, dtype=np.float32)).reshape(P_PATCH, 1)
    id128 = np.eye(128, dtype=np.float32)
    return {"gmat": g, "emat": e, "pbase": pbase, "id128": id128}


def build_program(split_waits=True):
    nc = bacc.Bacc()
    x = nc.declare_dram_parameter("x", [SPC * SEQ, D], F32, isOutput=False)
    gmat = nc.declare_dram_parameter("gmat", [128, 5, P_PATCH], F32, isOutput=False)
    emat = nc.declare_dram_parameter("emat", [P_PATCH, KT, 128], F32, isOutput=False)
    pbase = nc.declare_dram_parameter("pbase", [P_PATCH, 1], F32, isOutput=False)
    id128 = nc.declare_dram_parameter("id128", [128, 128], F32, isOutput=False)
    out = nc.declare_dram_parameter("out", [SPC * OUT_SEQ, D], F32, isOutput=True)
    idxd = nc.dram_tensor("idxd", [SPC, P_PATCH * TOP_K, 1], U32)
    innerd = nc.dram_tensor("innerd", [SPC, N_HIGH, 1], F32)

    with tile.TileContext(nc) as tc:
        with (
            tc.tile_pool(name="consts", bufs=1) as consts,
            tc.tile_pool(name="lowp", bufs=4) as lowp,
            tc.tile_pool(name="lowc", bufs=2) as lowc,
            tc.tile_pool(name="highp", bufs=5) as highp,
            tc.tile_pool(name="gathp", bufs=2) as gathp,
            tc.tile_pool(name="scr", bufs=1) as scr,
            tc.tile_pool(name="small", bufs=10) as small,
            tc.tile_pool(name="accs", bufs=6) as accs,
            tc.tile_pool(name="tkp", bufs=6) as tkp,
            tc.tile_pool(name="psq", bufs=1, space="PSUM") as psq,
            tc.tile_pool(name="psqx", bufs=2, space="PSUM") as psqx,
            tc.tile_pool(name="psit", bufs=1, space="PSUM") as psit,
        ):
            g_sb = consts.tile([128, 5, P_PATCH], F32)
            nc.sync.dma_start(g_sb[:], gmat[:])
            e_sb = consts.tile([P_PATCH, KT, 128], F32)
            nc.sync.dma_start(e_sb[:], emat[:])
            pbase_sb = consts.tile([P_PATCH, 1], F32)
            nc.sync.dma_start(pbase_sb[:], pbase[:])
            id_sb = consts.tile([128, 128], F32)
            nc.sync.dma_start(id_sb[:], id128[:])

            scr_act = scr.tile([128, D], F32)  # ACT throwaway output
            scr_dve = scr.tile([128, D], F32)  # DVE reduce throwaway

            psum_qs = {}
            q_sbs = {}
            low_tiles = {}
            high_tiles = {}
            ss_ts = {}
            dots_ts = {}

            def emit_low_loads(s):
                x0 = s * SEQ
                la = lowp.tile([128, 2, 1024], F32)
                nc.sync.dma_start(
                    la[:], x[x0 : x0 + 256, :].rearrange("(t p) d -> p t d", t=2)
                )
                lb = lowp.tile([128, 2, 1024], F32)
                nc.sync.dma_start(
                    lb[:], x[x0 + 256 : x0 + 512, :].rearrange("(t p) d -> p t d", t=2)
                )
                lc = lowc.tile([64, 1024], F32)
                nc.sync.dma_start(lc[:], x[x0 + 512 : x0 + 576, :])
                low_tiles[s] = (la, lb, lc)

            def _low_slice(s, t):
                la, lb, lc = low_tiles[s]
                if t < 2:
                    return la[:, t, :], 128
                if t < 4:
                    return lb[:, t - 2, :], 128
                return lc[:], 64

            def emit_low_tile(s, t):
                o0 = s * OUT_SEQ
                if t == 0:
                    psum_qs[s] = psq.tile([P_PATCH, D], F32, name="psum_q", tag="psum_q")
                sl, rows = _low_slice(s, t)
                ss = small.tile([128, 1], F32)
                nc.scalar.activation(
                    scr_act[:rows], sl, AF.Square, accum_out=ss[:rows]
                )
                nrm = small.tile([128, 1], F32)
                nc.scalar.activation(nrm[:rows], ss[:rows], AF.Sqrt)
                rn = small.tile([128, 1], F32)
                nc.vector.reciprocal(rn[:rows], nrm[:rows])
                # fold 1/norm into the tiny G slice so the q matmul can
                # consume the raw tile without waiting for the big rescale
                gsc = small.tile([128, P_PATCH], F32)
                nc.vector.tensor_scalar_mul(gsc[:rows], g_sb[:rows, t, :], rn[:rows])
                for h in range(2):
                    nc.tensor.matmul(
                        psum_qs[s][:, h * 512 : (h + 1) * 512],
                        lhsT=gsc[:rows],
                        rhs=sl[:, h * 512 : (h + 1) * 512],
                        start=(t == 0),
                        stop=(t == 4),
                    )
                nc.vector.tensor_scalar_mul(sl, sl, rn[:rows])
                nc.sync.dma_start(out[o0 + t * 128 : o0 + t * 128 + rows, :], sl)

            def emit_low_finish(s):
                q_sbs[s] = accs.tile([P_PATCH, D], F32, name="q_sb", tag="q_sb")
                nc.scalar.activation(q_sbs[s][:], psum_qs[s][:], AF.Copy)

            def emit_alloc_acc(s):
                ss_ts[s] = accs.tile([128, KT], F32, name="ss_t", tag="ss_t")
                dots_ts[s] = accs.tile([128, KT], F32, name="dots_t", tag="dots_t")

            def emit_high_load(s, b):
                x0 = s * SEQ + N_LOW + b * (HB * 128)
                ht = highp.tile([128, HB, 1024], F32)
                nc.sync.dma_start(
                    ht[:], x[x0 : x0 + HB * 128, :].rearrange("(t p) d -> p t d", t=HB)
                )
                high_tiles[(s, b)] = ht

            def emit_high_ss(s, k):
                ht = high_tiles[(s, k // HB)]
                nc.scalar.activation(
                    scr_act[:], ht[:, k % HB, :], AF.Square,
                    accum_out=ss_ts[s][:, k : k + 1],
                )

            def emit_high_dot(s, k):
                ht = high_tiles[(s, k // HB)]
                pqx = psqx.tile([128, D], F32)
                for h in range(2):
                    nc.tensor.matmul(
                        pqx[:, h * 512 : (h + 1) * 512],
                        lhsT=e_sb[:, k, :],
                        rhs=q_sbs[s][:, h * 512 : (h + 1) * 512],
                        start=True,
                        stop=True,
                    )
                nc.vector.scalar_tensor_tensor(
                    scr_dve[:],
                    ht[:, k % HB, :],
                    0.0,
                    pqx[:],
                    op0=OP.bypass,
                    op1=OP.mult,
                    accum_out=dots_ts[s][:, k : k + 1],
                )

            def emit_topk(s):
                ssq = tkp.tile([128, KT], F32)
                nc.scalar.activation(ssq[:], ss_ts[s][:], AF.Sqrt)
                rn_t = tkp.tile([128, KT], F32)
                nc.vector.reciprocal(rn_t[:], ssq[:])
                inner_t = tkp.tile([128, KT], F32)
                nc.vector.tensor_mul(inner_t[:], dots_ts[s][:], rn_t[:])
                pit = psit.tile([KT, 128], F32)
                nc.tensor.transpose(pit[:], inner_t[:], id_sb[:])
                it18 = tkp.tile([KT, 128], F32)
                nc.scalar.activation(it18[:], pit[:], AF.Copy)
                nc.sync.dma_start(
                    innerd[s].rearrange("(k i) one -> k (i one)", k=KT), it18[:]
                )
                in96 = tkp.tile([P_PATCH, GH], F32)
                nc.sync.dma_start(
                    in96[:], innerd[s].rearrange("(p g) one -> p (g one)", p=P_PATCH)
                )
                mx8 = small.tile([P_PATCH, TOP_K], F32)
                nc.vector.max(out=mx8[:], in_=in96[:])
                ix8 = small.tile([P_PATCH, TOP_K], U32)
                nc.vector.max_index(out=ix8[:], in_max=mx8[:], in_values=in96[:])
                ixf = small.tile([P_PATCH, TOP_K], F32)
                nc.vector.tensor_copy(ixf[:], ix8[:])
                ixg = small.tile([P_PATCH, TOP_K], F32)
                nc.vector.tensor_scalar(
                    ixg[:], ixf[:], pbase_sb[:], float(s * SEQ),
                    op0=OP.add, op1=OP.add,
                )
                ixu = small.tile([P_PATCH, TOP_K], U32)
                nc.vector.tensor_copy(ixu[:], ixg[:])
                nc.sync.dma_start(
                    idxd[s].rearrange("(a b) c -> a (b c)", a=P_PATCH), ixu[:]
                )

            def emit_gather(s, gi):
                o0 = s * OUT_SEQ
                rows = 128 if gi == 0 else 64
                base = gi * 128
                ixcol = small.tile([128, 1], U32)
                nc.sync.dma_start(ixcol[:rows], idxd[s, base : base + rows, :])
                gt = gathp.tile([128, D], F32)
                nc.gpsimd.indirect_dma_start(
                    out=gt[:rows],
                    out_offset=None,
                    in_=x[:],
                    in_offset=IndirectOffsetOnAxis(ap=ixcol[:rows], axis=0),
                )
                ssg = small.tile([128, 1], F32)
                nc.scalar.activation(
                    scr_act[:rows], gt[:rows], AF.Square, accum_out=ssg[:rows]
                )
                nrg = small.tile([128, 1], F32)
                nc.scalar.activation(nrg[:rows], ssg[:rows], AF.Sqrt)
                rg = small.tile([128, 1], F32)
                nc.vector.reciprocal(rg[:rows], nrg[:rows])
                nc.vector.tensor_scalar_mul(gt[:rows], gt[:rows], rg[:rows])
                nc.sync.dma_start(
                    out[o0 + N_LOW + base : o0 + N_LOW + base + rows, :], gt[:rows]
                )

            # ---- schedule ----
            # sample 0: low loads + first high blocks prefetch
            emit_low_loads(0)
            emit_high_load(0, 0)
            emit_high_load(0, 1)
            for t in range(5):
                emit_low_tile(0, t)
            emit_alloc_acc(0)
            emit_low_finish(0)
            # high loop sample 0; keep 2 blocks of lookahead; ride sample 1's
            # low phase inside it
            for k in range(KT):
                b = k // HB
                if k % HB == 0 and b + 2 < NHB:
                    emit_high_load(0, b + 2)
                emit_high_ss(0, k)
                emit_high_dot(0, k)
                if k == 4:
                    emit_low_loads(1)
                if k in (8, 10, 12, 14, 16):
                    emit_low_tile(1, (k - 8) // 2)
            emit_alloc_acc(1)
            emit_high_load(1, 0)
            emit_high_load(1, 1)
            emit_low_finish(1)
            emit_topk(0)
            for k in range(KT):
                b = k // HB
                if k % HB == 0 and b + 2 < NHB:
                    emit_high_load(1, b + 2)
                emit_high_ss(1, k)
                emit_high_dot(1, k)
                if k == 4:
                    emit_gather(0, 0)
                elif k == 8:
                    emit_gather(0, 1)
            emit_topk(1)
            emit_gather(1, 0)
            emit_gather(1, 1)
    nc.finalize()
    if split_waits:
        _split_excess_waits(nc)
    return nc


_CACHED = {}


def _get_program():
    if "nc" not in _CACHED:
        _CACHED["nc"] = build_program()
    return _CACHED["nc"]


def kernel(x: np.ndarray) -> np.ndarray:
    assert x.shape == (BSZ, SEQ, D), x.shape
    x = np.ascontiguousarray(x, dtype=np.float32)
    consts = host_constants()
    shards = x.reshape(N_CORES, SPC * SEQ, D)
    in_maps = [dict(consts, x=shards[i]) for i in range(N_CORES)]
    nc = _get_program()
    res = run_bass_kernel_spmd(nc, in_maps, core_ids=list(range(N_CORES)))
    outs = [res.results[i]["out"].reshape(SPC, OUT_SEQ, D) for i in range(N_CORES)]
    return np.concatenate(outs, axis=0).astype(np.float32)
